# revision 14
# baseline (speedup 1.0000x reference)
"""GAT message-passing layer (gnn_message_passing) on 8 Trainium2 NeuronCores.

Strategy (dst-sharded graph parallelism):
  - Nodes are sharded across 8 cores by destination. Core k owns dst nodes
    [k*6250, (k+1)*6250). Inputs to each core are node-rotated so its own
    shard is rows [0, SHARD) -- this keeps the SPMD program identical on
    every core while all per-core offsets stay uniform.
  - Phase A (per core): h = x @ W_gat for ALL nodes (bf16), plus
    a_src/a_dst = x @ v_fold, plus the feature_transform path
    (BN folded into the weights on the host) for the core's own shard.
    h rows (with a_src appended) form a DRAM gather table `htab`
    (row g+1 = local node g; row 0 / row npad+1 are -inf-logit dummies).
  - Phase B: edges (dst-sorted, grouped into 128-dst blocks, padded to a
    uniform number of 128-edge groups per block) are processed with bulk
    dma_gather of h[src]. dma_gather indices are signed int16, so the
    table is split at row 32768: per block, edges whose src-row < 32768
    fill the first G_A groups (gathered from the table base), the rest
    fill G_B groups (gathered from a +32768-row offset view).
    Segment softmax uses the exp/sum formulation (max-subtraction is
    unnecessary: logits are O(1) and denom >= exp(leaky(self_logit)) > 0).
    Per-edge one-hot selection matrices (built with one DVE compare
    against an iota) turn the per-dst segment sum into PE matmuls
    accumulating in PSUM; the softmax denominator rides along as 4 extra
    matmul columns. Normalisation + feature_transform happen in a
    per-block epilogue.

Self-contained: hardcodes the problem shapes from the task spec.
"""

import math
from dataclasses import dataclass

import numpy as np
import ml_dtypes

import concourse.bass as bass
import concourse.tile as tile
from concourse import mybir, bacc
from concourse.bass_utils import run_bass_kernel_spmd

BF16 = ml_dtypes.bfloat16

IN_C = 256
OUT_C = 256
HEADS = 4
C = OUT_C // HEADS
BN_EPS = 1e-5
P = 128
ROW = 384          # htab row stride (256 h + 4 a_src + 4 a_dst + pad);
                   # must be a multiple of 128 bf16 (256B) for dma_gather
SB = 2             # dst-blocks per dma_gather superblock


@dataclass
class Cfg:
    n: int            # number of nodes
    ncores: int       # SPMD width
    g_a: int          # groups (128 edge slots) per block from table part A
    g_b: int          # groups per block from table part B

    @property
    def shard(self):
        assert self.n % self.ncores == 0
        return self.n // self.ncores

    @property
    def nblk(self):
        return math.ceil(self.shard / P)

    @property
    def npad(self):  # phase-A node tiles * P (covers all nodes)
        return math.ceil((self.n + 1) / P) * P

    @property
    def b0(self):    # table split row (part B = rows [b0, ...))
        nrows = self.npad + 2 * P
        return 32768 if nrows > 32768 else nrows // 2

    @property
    def dh(self):    # dummy-high table row (for part-B padding)
        return self.npad + 1

    @property
    def gpb(self):
        return self.g_a + self.g_b


def build_program(cfg: Cfg):
    """Build the SPMD Bass program (same NEFF for all cores)."""
    fp32 = mybir.dt.float32
    bf16 = mybir.dt.bfloat16
    int16 = mybir.dt.int16
    uint8 = mybir.dt.uint8

    nc = bacc.Bacc("TRN2", target_bir_lowering=False, debug=False)

    NGT = cfg.nblk * cfg.gpb           # total groups per core
    ntile = cfg.npad // P              # phase-A node tiles
    NROWS = cfg.npad + 2 * P           # htab rows (node rows at +1, dummies)
    assert cfg.dh < NROWS and cfg.b0 < NROWS
    assert cfg.n + 1 < cfg.b0 + 32768 and cfg.dh - cfg.b0 < 32768

    xT = nc.dram_tensor("xT", [IN_C, cfg.npad], bf16, kind="ExternalInput").ap()
    W = nc.dram_tensor("W", [IN_C, OUT_C], bf16, kind="ExternalInput").ap()
    vf_d = nc.dram_tensor("vf", [IN_C, 2 * HEADS], bf16, kind="ExternalInput").ap()
    wft = nc.dram_tensor("wft", [IN_C, OUT_C], bf16, kind="ExternalInput").ap()
    cft = nc.dram_tensor("cft", [1, OUT_C], fp32, kind="ExternalInput").ap()
    biasg = nc.dram_tensor("biasg", [1, OUT_C], fp32, kind="ExternalInput").ap()
    idxA_d = nc.dram_tensor("idxA", [P, cfg.nblk * cfg.g_a * 8], int16,
                            kind="ExternalInput").ap()
    idxB_d = nc.dram_tensor("idxB", [P, cfg.nblk * cfg.g_b * 8], int16,
                            kind="ExternalInput").ap()
    dmP = nc.dram_tensor("dmP", [P, NGT], uint8, kind="ExternalInput").ap()
    dmF = nc.dram_tensor("dmF", [cfg.nblk, cfg.gpb * P], uint8,
                         kind="ExternalInput").ap()
    htab = nc.dram_tensor("htab", [NROWS, ROW], bf16, kind="Internal").ap()
    out = nc.dram_tensor("out", [cfg.nblk * P, OUT_C], fp32,
                         kind="ExternalOutput").ap()

    with tile.TileContext(nc) as tc:
        # ---- persistent constants / state -------------------------------
        with (
            tc.tile_pool(name="const", bufs=1) as cpool,
            tc.tile_pool(name="state", bufs=1) as spool,
        ):
            wt = cpool.tile([P, 2, OUT_C + 8], bf16, tag="wt")
            nc.sync.dma_start(out=wt[:, 0, 0:OUT_C], in_=W[0:P, :])
            nc.sync.dma_start(out=wt[:, 1, 0:OUT_C], in_=W[P:2 * P, :])
            nc.sync.dma_start(out=wt[:, 0, OUT_C:], in_=vf_d[0:P, :])
            nc.sync.dma_start(out=wt[:, 1, OUT_C:], in_=vf_d[P:2 * P, :])
            wf = cpool.tile([P, 2, OUT_C], bf16, tag="wf")
            nc.sync.dma_start(out=wf[:, 0, :], in_=wft[0:P, :])
            nc.sync.dma_start(out=wf[:, 1, :], in_=wft[P:2 * P, :])
            cftb = cpool.tile([P, OUT_C], fp32, tag="cftb")
            nc.sync.dma_start(out=cftb[:], in_=cft[0:1, :].to_broadcast([P, OUT_C]))
            biasb = cpool.tile([P, OUT_C], fp32, tag="biasb")
            nc.sync.dma_start(out=biasb[:],
                              in_=biasg[0:1, :].to_broadcast([P, OUT_C]))

            iota_f = cpool.tile([P, P], uint8, tag="iota_f")
            nc.gpsimd.iota(iota_f[:], [[1, P]], channel_multiplier=0,
                           allow_small_or_imprecise_dtypes=True)
            iota_p = cpool.tile([P, 1], uint8, tag="iota_p")
            nc.gpsimd.iota(iota_p[:], [[0, 1]], channel_multiplier=1,
                           allow_small_or_imprecise_dtypes=True)
            # dummy htab row: h = 0, a_src = -200 (leaky scales by 0.2 -> exp(-40) ~ 0)
            dmy = cpool.tile([1, OUT_C + 8], bf16, tag="dmy")
            nc.gpsimd.memset(dmy[:], 0.0)
            nc.gpsimd.memset(dmy[:, OUT_C:OUT_C + 4], -200.0)

            idxA = spool.tile([P, cfg.nblk * cfg.g_a * 8], int16, tag="idxA")
            nc.sync.dma_start(out=idxA[:], in_=idxA_d[:, :])
            idxB = spool.tile([P, cfg.nblk * cfg.g_b * 8], int16, tag="idxB")
            nc.sync.dma_start(out=idxB[:], in_=idxB_d[:, :])
            dmPt = spool.tile([P, NGT], uint8, tag="dmPt")
            nc.sync.dma_start(out=dmPt[:], in_=dmP[:, :])

            adst = spool.tile([P, cfg.nblk * HEADS], bf16, tag="adst")
            transf = spool.tile([P, cfg.nblk * OUT_C], fp32, tag="transf")

            # ---- phase A: h table + a_src/a_dst + feature_transform -----
            CH = 4  # node subtiles per DMA chunk
            with (
                tc.tile_pool(name="xa", bufs=3) as xa,
                tc.tile_pool(name="hps", bufs=4, space="PSUM") as hpp,
                tc.tile_pool(name="ftps", bufs=2, space="PSUM") as ftp,
                tc.tile_pool(name="hsb", bufs=4) as hsb,
                tc.tile_pool(name="ftsb", bufs=2) as ftsb,
            ):
                nc.sync.dma_start(out=htab[0:1, 0:OUT_C + 8], in_=dmy[:])
                nc.sync.dma_start(out=htab[cfg.dh:cfg.dh + 1, 0:OUT_C + 8],
                                  in_=dmy[:])
                nchunk = math.ceil(ntile / CH)
                for ci in range(nchunk):
                    ch = min(CH, ntile - ci * CH)
                    xt = xa.tile([P, 2, CH * P], bf16, tag="xt")
                    c0 = ci * CH * P
                    nc.sync.dma_start(out=xt[:, 0, 0:ch * P],
                                      in_=xT[0:P, c0:c0 + ch * P])
                    nc.sync.dma_start(out=xt[:, 1, 0:ch * P],
                                      in_=xT[P:2 * P, c0:c0 + ch * P])
                    for st in range(ch):
                        t = ci * CH + st
                        l0 = xt[:, 0, st * P:(st + 1) * P]
                        l1 = xt[:, 1, st * P:(st + 1) * P]
                        hps = hpp.tile([P, OUT_C + 8], fp32, tag="hps")
                        nc.tensor.matmul(hps[:], l0, wt[:, 0, :],
                                         start=True, stop=False)
                        nc.tensor.matmul(hps[:], l1, wt[:, 1, :],
                                         start=False, stop=True)
                        hs = hsb.tile([P, OUT_C + 8], bf16, tag="hs")
                        # split the big cast across DVE and ACT
                        nc.vector.tensor_copy(out=hs[:, 0:P], in_=hps[:, 0:P])
                        nc.scalar.copy(out=hs[:, P:OUT_C + 8],
                                       in_=hps[:, P:OUT_C + 8])
                        if t < cfg.nblk:
                            nc.vector.tensor_copy(
                                out=adst[:, t * HEADS:(t + 1) * HEADS],
                                in_=hps[:, OUT_C + HEADS:OUT_C + 2 * HEADS])
                            fps = ftp.tile([P, OUT_C], fp32, tag="fps")
                            nc.tensor.matmul(fps[:], l0, wf[:, 0, :],
                                             start=True, stop=False)
                            nc.tensor.matmul(fps[:], l1, wf[:, 1, :],
                                             start=False, stop=True)
                            tmp = ftsb.tile([P, OUT_C], fp32, tag="tmp")
                            nc.vector.tensor_tensor(
                                out=tmp[:], in0=fps[:], in1=cftb[:],
                                op=mybir.AluOpType.add)
                            # relu(tmp) + bias_gat
                            nc.vector.scalar_tensor_tensor(
                                out=transf[:, t * OUT_C:(t + 1) * OUT_C],
                                in0=tmp[:], scalar=0.0, in1=biasb[:],
                                op0=mybir.AluOpType.max,
                                op1=mybir.AluOpType.add)
                        nc.sync.dma_start(
                            out=htab[t * P + 1:(t + 1) * P + 1, 0:OUT_C + 8],
                            in_=hs[:])

            # DRAM RAW deps (htab writes -> gathers) are not tracked by
            # Tile; fence the phases explicitly.
            tc.strict_bb_all_engine_barrier()

            # ---- phase B: edge processing -------------------------------
            with (
                tc.tile_pool(name="gA", bufs=3) as gAp,
                tc.tile_pool(name="gB", bufs=3) as gBp,
                tc.tile_pool(name="dmf", bufs=2) as dmfp,
                tc.tile_pool(name="selE", bufs=3) as selEp,
                tc.tile_pool(name="selT", bufs=3) as selTp,
                tc.tile_pool(name="msg", bufs=3) as msgp,
                tc.tile_pool(name="logit", bufs=3) as logp,
                tc.tile_pool(name="ealp", bufs=3) as ealpp,
                tc.tile_pool(name="aggps", bufs=2, space="PSUM") as aggp,
                tc.tile_pool(name="adps", bufs=3, space="PSUM") as adpp,
                tc.tile_pool(name="outsb", bufs=2) as outp,
                tc.tile_pool(name="rden", bufs=2) as rdp,
            ):
                MAXG = 16  # dma_gather cap: 2048 indices per call
                for b in range(cfg.nblk):
                    gtA = gAp.tile([P, cfg.g_a, ROW], bf16, tag="gtA")
                    for c0 in range(0, cfg.g_a, MAXG):
                        gc = min(MAXG, cfg.g_a - c0)
                        nc.gpsimd.dma_gather(
                            out_ap=gtA[:, c0:c0 + gc, :], in_ap=htab[:, :],
                            idxs_ap=idxA[:, (b * cfg.g_a + c0) * 8:
                                         (b * cfg.g_a + c0 + gc) * 8],
                            num_idxs=gc * P, num_idxs_reg=gc * P,
                            elem_size=ROW, single_packet=(gc * P <= 1024))
                    gtB = gBp.tile([P, cfg.g_b, ROW], bf16, tag="gtB")
                    for c0 in range(0, cfg.g_b, MAXG):
                        gc = min(MAXG, cfg.g_b - c0)
                        nc.gpsimd.dma_gather(
                            out_ap=gtB[:, c0:c0 + gc, :],
                            in_ap=htab[cfg.b0:, :],
                            idxs_ap=idxB[:, (b * cfg.g_b + c0) * 8:
                                         (b * cfg.g_b + c0 + gc) * 8],
                            num_idxs=gc * P, num_idxs_reg=gc * P,
                            elem_size=ROW, single_packet=(gc * P <= 1024))
                    if True:
                        dmft = dmfp.tile([P, cfg.gpb * P], uint8, tag="dmft")
                        nc.sync.dma_start(
                            out=dmft[:],
                            in_=dmF[b:b + 1, :].to_broadcast([P, cfg.gpb * P]))
                        agg = aggp.tile([P, OUT_C + HEADS], fp32, tag="agg")
                        # macrotiles: A-part groups then B-part groups
                        mts = []
                        for a0 in range(0, cfg.g_a, 4):
                            mts.append((gtA, a0, a0, min(4, cfg.g_a - a0)))
                        for e0 in range(0, cfg.g_b, 4):
                            mts.append((gtB, e0, cfg.g_a + e0,
                                        min(4, cfg.g_b - e0)))
                        for mi, (gt, gc0, gl0, gn) in enumerate(mts):
                            selE = selEp.tile([P, 4, P], bf16, tag="selE")
                            nc.vector.tensor_tensor(
                                out=selE[:, 0:gn, :],
                                in0=dmPt[:, b * cfg.gpb + gl0:
                                         b * cfg.gpb + gl0 + gn]
                                    .unsqueeze(-1).to_broadcast([P, gn, P]),
                                in1=iota_f[:, :].unsqueeze(1)
                                    .to_broadcast([P, gn, P]),
                                op=mybir.AluOpType.is_equal)
                            selT = selTp.tile([P, 4, P], bf16, tag="selT")
                            nc.vector.tensor_tensor(
                                out=selT[:, 0:gn, :],
                                in0=iota_p[:, 0:1].unsqueeze(1)
                                    .to_broadcast([P, gn, P]),
                                in1=dmft[:, gl0 * P:(gl0 + gn) * P]
                                    .rearrange("p (g e) -> p g e", g=gn),
                                op=mybir.AluOpType.is_equal)
                            adp = adpp.tile([P, 4 * HEADS], fp32, tag="adp")
                            for g in range(gn):
                                nc.tensor.matmul(
                                    adp[:, g * HEADS:(g + 1) * HEADS],
                                    selT[:, g, :],
                                    adst[:, b * HEADS:(b + 1) * HEADS],
                                    start=True, stop=True)
                            logit = logp.tile([P, 4 * HEADS], fp32, tag="logit")
                            nc.vector.tensor_tensor(
                                out=logit[:, 0:gn * HEADS]
                                    .rearrange("p (g h) -> p g h", h=HEADS),
                                in0=adp[:, 0:gn * HEADS]
                                    .rearrange("p (g h) -> p g h", h=HEADS),
                                in1=gt[:, gc0:gc0 + gn, OUT_C:OUT_C + HEADS],
                                op=mybir.AluOpType.add)
                            # leaky_relu(x) = max(0.2*x, x)
                            nc.vector.scalar_tensor_tensor(
                                out=logit[:, 0:gn * HEADS],
                                in0=logit[:, 0:gn * HEADS], scalar=0.2,
                                in1=logit[:, 0:gn * HEADS],
                                op0=mybir.AluOpType.mult,
                                op1=mybir.AluOpType.max)
                            ealp = ealpp.tile([P, 4 * HEADS], bf16, tag="ealp")
                            nc.scalar.activation(
                                out=ealp[:, 0:gn * HEADS],
                                in_=logit[:, 0:gn * HEADS],
                                func=mybir.ActivationFunctionType.Exp)
                            msg = msgp.tile([P, 4, OUT_C + HEADS], bf16,
                                            tag="msg")
                            nc.vector.tensor_tensor(
                                out=msg[:, 0:gn, 0:OUT_C]
                                    .rearrange("p g (h c) -> p g h c", h=HEADS),
                                in0=gt[:, gc0:gc0 + gn, 0:OUT_C]
                                    .rearrange("p g (h c) -> p g h c", h=HEADS),
                                in1=ealp[:, 0:gn * HEADS]
                                    .rearrange("p (g h) -> p g h", g=gn)
                                    .unsqueeze(-1)
                                    .to_broadcast([P, gn, HEADS, C]),
                                op=mybir.AluOpType.mult)
                            nc.vector.tensor_copy(
                                out=msg[:, 0:gn, OUT_C:OUT_C + HEADS],
                                in_=ealp[:, 0:gn * HEADS]
                                    .rearrange("p (g h) -> p g h", g=gn))
                            for g in range(gn):
                                nc.tensor.matmul(
                                    agg[:],
                                    selE[:, g, :],
                                    msg[:, g, :],
                                    start=(gl0 + g == 0),
                                    stop=(gl0 + g == cfg.gpb - 1))
                        rd = rdp.tile([P, HEADS], fp32, tag="rd")
                        nc.vector.reciprocal(out=rd[:],
                                             in_=agg[:, OUT_C:OUT_C + HEADS])
                        ob = outp.tile([P, OUT_C], fp32, tag="ob")
                        nc.vector.tensor_tensor(
                            out=ob[:].rearrange("p (h c) -> p h c", h=HEADS),
                            in0=agg[:, 0:OUT_C].rearrange("p (h c) -> p h c",
                                                          h=HEADS),
                            in1=rd[:].unsqueeze(-1).to_broadcast([P, HEADS, C]),
                            op=mybir.AluOpType.mult)
                        nc.vector.tensor_tensor(
                            out=ob[:], in0=ob[:],
                            in1=transf[:, b * OUT_C:(b + 1) * OUT_C],
                            op=mybir.AluOpType.add)
                        nc.sync.dma_start(out=out[b * P:(b + 1) * P, :],
                                          in_=ob[:])
    nc.compile()
    return nc


def _wrap16(idx_flat):
    """dma_gather index layout: idx j -> partition j%16, col j//16,
    replicated across the 8 GPSIMD core groups."""
    n = len(idx_flat)
    assert n % 16 == 0
    a = np.asarray(idx_flat, np.int64).reshape(n // 16, 16).T.astype(np.int16)
    return np.ascontiguousarray(np.tile(a, (8, 1)))


def compute_gab(cfg0, dst_all, src_all, b0_split, n):
    """Per-(core, block) group needs for table parts A and B."""
    ga, gb = 1, 1
    for k in range(cfg0.ncores):
        base = k * cfg0.shard
        m = (dst_all >= base) & (dst_all < base + cfg0.shard)
        dstK = dst_all[m] - base
        rowK = ((src_all[m] - base) % n) + 1
        isA = rowK < b0_split
        cntA = np.bincount(dstK[isA] // P, minlength=cfg0.nblk)
        cntB = np.bincount(dstK[~isA] // P, minlength=cfg0.nblk)
        ga = max(ga, int(np.ceil(cntA.max() / P)))
        gb = max(gb, int(np.ceil(cntB.max() / P)))
    return ga, gb


def host_prep(cfg: Cfg, x, edge_index, W_gat, att_src, att_dst, bias_gat,
              w_ft, b_ft, bn_gamma, bn_beta, bn_mean, bn_var):
    """Pure index/layout preprocessing + weight folding. Returns in_maps."""
    n = cfg.n
    x = np.asarray(x, dtype=np.float32)
    ei = np.asarray(edge_index)
    src_all = np.concatenate([ei[0], np.arange(n, dtype=ei.dtype)]).astype(np.int64)
    dst_all = np.concatenate([ei[1], np.arange(n, dtype=ei.dtype)]).astype(np.int64)

    # weight folding (host-side, input-independent)
    W64 = np.asarray(W_gat, dtype=np.float64)
    vsrc = np.einsum("ihc,hc->ih", W64.reshape(IN_C, HEADS, C),
                     np.asarray(att_src, dtype=np.float64))
    vdst = np.einsum("ihc,hc->ih", W64.reshape(IN_C, HEADS, C),
                     np.asarray(att_dst, dtype=np.float64))
    vf = np.concatenate([vsrc, vdst], axis=1)  # [IN_C, 8]
    s = np.asarray(bn_gamma, np.float64) / np.sqrt(
        np.asarray(bn_var, np.float64) + BN_EPS)
    wft_f = np.asarray(w_ft, np.float64) * s[None, :]
    cft = (np.asarray(b_ft, np.float64) * s
           + np.asarray(bn_beta, np.float64)
           - np.asarray(bn_mean, np.float64) * s)

    xT_g = np.ascontiguousarray(x.T).astype(BF16)  # [IN_C, n]

    shared = {
        "W": np.asarray(W_gat, np.float32).astype(BF16),
        "vf": vf.astype(np.float32).astype(BF16),
        "wft": wft_f.astype(np.float32).astype(BF16),
        "cft": cft.astype(np.float32)[None, :],
        "biasg": np.asarray(bias_gat, np.float32)[None, :],
    }

    in_maps = []
    NSA = cfg.nblk * cfg.g_a * P
    NSB = cfg.nblk * cfg.g_b * P
    for k in range(cfg.ncores):
        base = k * cfg.shard
        m = (dst_all >= base) & (dst_all < base + cfg.shard)
        dstK = (dst_all[m] - base).astype(np.int64)
        rowK = ((src_all[m] - base) % n).astype(np.int64) + 1  # table rows
        isA = rowK < cfg.b0
        blk = dstK // P
        dmod = dstK - blk * P

        srcA = np.zeros(NSA, np.int64)          # pad -> dummy-low row 0
        dmA = np.zeros(NSA, np.int64)
        srcB = np.full(NSB, cfg.dh - cfg.b0, np.int64)  # pad -> dummy-high
        dmB = np.zeros(NSB, np.int64)
        for part, sel in (("A", isA), ("B", ~isA)):
            r = rowK[sel]
            bl = blk[sel]
            dm = dmod[sel]
            order = np.argsort(bl, kind="stable")
            r, bl, dm = r[order], bl[order], dm[order]
            cnt = np.bincount(bl, minlength=cfg.nblk)
            starts = np.cumsum(cnt) - cnt
            pos = np.arange(len(bl)) - np.repeat(starts, cnt)
            if part == "A":
                slot = bl * (cfg.g_a * P) + pos
                srcA[slot] = r
                dmA[slot] = dm
            else:
                slot = bl * (cfg.g_b * P) + pos
                srcB[slot] = r - cfg.b0
                dmB[slot] = dm

        idxA = _wrap16(srcA)
        idxB = _wrap16(srcB)
        # dst_mod layout over the combined per-block groups [A | B]
        dmAll = np.zeros((cfg.nblk, cfg.gpb, P), np.uint8)
        dmAll[:, 0:cfg.g_a, :] = dmA.reshape(cfg.nblk, cfg.g_a, P)
        dmAll[:, cfg.g_a:, :] = dmB.reshape(cfg.nblk, cfg.g_b, P)
        dmPa = np.ascontiguousarray(
            dmAll.reshape(cfg.nblk * cfg.gpb, P).T)
        dmFa = np.ascontiguousarray(dmAll.reshape(cfg.nblk, cfg.gpb * P))

        xTr = np.zeros((IN_C, cfg.npad), dtype=BF16)
        xTr[:, :n - base] = xT_g[:, base:]
        xTr[:, n - base:n] = xT_g[:, :base]
        in_maps.append(dict(shared, xT=xTr, idxA=idxA, idxB=idxB,
                            dmP=dmPa, dmF=dmFa))
    return in_maps


def make_cfg(n, edge_index, ncores=8):
    ei = np.asarray(edge_index)
    cfg0 = Cfg(n=n, ncores=ncores, g_a=1, g_b=1)
    src_all = np.concatenate(
        [ei[0], np.arange(n, dtype=ei.dtype)]).astype(np.int64)
    dst_all = np.concatenate(
        [ei[1], np.arange(n, dtype=ei.dtype)]).astype(np.int64)
    ga, gb = compute_gab(cfg0, dst_all, src_all, cfg0.b0, n)
    return Cfg(n=n, ncores=ncores, g_a=ga, g_b=gb)


def run_cfg(cfg: Cfg, inputs: dict, trace: bool = False, nc=None):
    """Build + run; returns (full_output [n, OUT_C] f32, BassKernelResults)."""
    in_maps = host_prep(
        cfg, inputs["x"], inputs["edge_index"], inputs["W_gat"],
        inputs["att_src"], inputs["att_dst"], inputs["bias_gat"],
        inputs["w_ft"], inputs["b_ft"], inputs["bn_gamma"],
        inputs["bn_beta"], inputs["bn_mean"], inputs["bn_var"])
    if nc is None:
        nc = build_program(cfg)
    res = run_bass_kernel_spmd(nc, in_maps, core_ids=list(range(cfg.ncores)),
                               trace=trace)
    out = np.concatenate(
        [res.results[k]["out"][:cfg.shard] for k in range(cfg.ncores)], axis=0)
    return out.astype(np.float32), res


def kernel(**inputs) -> np.ndarray:
    n = inputs["x"].shape[0]
    cfg = make_cfg(n, inputs["edge_index"])
    out, _ = run_cfg(cfg, inputs)
    return out
